# revision 8
# baseline (speedup 1.0000x reference)
"""Trainium2 Bass kernel for LocalWindowAttention.

Computation (per batch b):
    tokens = xb[b].mean(-1)                    # [NB, C]
    Q/K/V  = tokens @ W{q,k,v} + pos           # [NB, D]
    scores = window-attn over NB (win=9, clamped) with scale 1/sqrt(D)
    ctx    = softmax(scores) @ V_window        # [NB, D]
    out    = xb[b] + (ctx @ Wo)[..., None]     # broadcast over T

Strategy: data-parallel over B across 8 NeuronCores (1 batch/core).
Per core, a single NEFF does:
  Phase A: stream xb (bf16) from HBM, T-sum on DVE -> tokens (bf16),
           PE-transpose -> tokensT.
  Proj:    Q_T/K_T (d-major) and V (n-major) via PE matmuls (bf16),
           + pos adds on DVE.
  Attn:    per 128-row block: banded scores via PE against a 136-wide
           K_T window, clamped-window softmax done exactly via an
           additive log-multiplicity mask + ACT exp (accum_out gives
           the denominator), attn transposed on PE, ctx_T and out_tok
           via PE matmuls.
  Phase B: re-stream xb (fp32), DVE broadcast-add of out_tok over T,
           store result.

The clamped gather of the reference (indices clipped at the edges,
duplicating edge rows) is reproduced exactly by adding log(multiplicity)
to the score of each unique column inside the softmax.
"""

import numpy as np
import ml_dtypes

import concourse.bass as bass
import concourse.mybir as mybir
import concourse.tile as tile
import concourse.bacc as bacc
from concourse import masks as cmasks
from concourse.bass_utils import run_bass_kernel_spmd

# Problem shapes (hardcoded per contest rules)
B, NB, C, T = 8, 1024, 1024, 32
D = 1024
WIN, HALF = 9, 4
P = 128                       # partitions
NBLK = NB // P                # 8 row blocks
CCH = C // P                  # 8 c-chunks
DCH = D // P                  # 8 d-chunks
WWIN = 192                    # window columns per block (32-aligned segs)
SCALE = 1.0 / np.sqrt(D)      # 1/32
A_CC = 256                    # phase-A c-chunk per stream tile
B_CC = 128                    # phase-B c-chunk per stream tile

F32 = mybir.dt.float32
BF16 = mybir.dt.bfloat16

N_CORES = 8


def _w0(i):
    """Window start for block i; chosen so all V-block segments are
    32-aligned (legal matmul tile_positions)."""
    return min(max(i * P - 32, 0), NB - WWIN)


def _build_masks():
    """Per-block additive masks [NBLK, P, WWIN] (pre-divided by SCALE):
    log(multiplicity) on in-band columns (reproduces the reference's
    clamped gather exactly), -1e30 elsewhere."""
    m = np.full((NBLK, P, WWIN), -1e30, np.float32)
    for i in range(NBLK):
        w0 = _w0(i)
        for r in range(P):
            n = i * P + r
            idx = np.clip(n - HALF + np.arange(WIN), 0, NB - 1)
            u, cnt = np.unique(idx, return_counts=True)
            m[i, r, u - w0] = np.log(cnt.astype(np.float64)) / SCALE
    return m


_MASKS = _build_masks()


def _segments(i):
    """V-block segments covering window [w0, w0+WWIN) for block i as
    (blk, p0, ln, cofs): rows [p0, p0+ln) of V block `blk` correspond to
    window columns [cofs, cofs+ln). All splits are 32-aligned so both
    the attn transposes and the banded matmuls get legal tile
    positions."""
    w0 = _w0(i)
    segs = []
    lo, hi = w0, w0 + WWIN
    for blk in range(NBLK):
        b0, b1 = blk * P, (blk + 1) * P
        s0, s1 = max(lo, b0), min(hi, b1)
        if s0 < s1:
            segs.append((blk, s0 - b0, s1 - s0, s0 - w0))
    return segs


def build_nc():
    nc = bacc.Bacc("TRN2", target_bir_lowering=False, debug=False,
                   num_devices=N_CORES)

    xb_f = nc.declare_dram_parameter("xb", [NB, C, T], F32, isOutput=False)
    xb_h = nc.declare_dram_parameter("xbh", [NB, C, T], BF16, isOutput=False)
    wq_d = nc.declare_dram_parameter("wq", [C, D], BF16, isOutput=False)
    wk_d = nc.declare_dram_parameter("wk", [C, D], BF16, isOutput=False)
    wv_d = nc.declare_dram_parameter("wv", [C, D], BF16, isOutput=False)
    wo_d = nc.declare_dram_parameter("wo", [D, C], BF16, isOutput=False)
    pos_d = nc.declare_dram_parameter("pos", [NB, D], BF16, isOutput=False)
    post_d = nc.declare_dram_parameter("post", [D, NB], BF16, isOutput=False)
    mask_d = nc.declare_dram_parameter("mask", [NBLK, P, WWIN], F32,
                                       isOutput=False)
    out_d = nc.declare_dram_parameter("out", [NB, C, T], F32, isOutput=True)

    with tile.TileContext(nc) as tc:
        _emit(nc, tc, xb_f, xb_h, wq_d, wk_d, wv_d, wo_d, pos_d, post_d,
              mask_d, out_d)

    nc.compile()
    return nc


def _emit(nc, tc, xb_f, xb_h, wq_d, wk_d, wv_d, wo_d, pos_d, post_d,
          mask_d, out_d):
    from contextlib import ExitStack

    with ExitStack() as ctx:
        const_pool = ctx.enter_context(tc.tile_pool(name="const", bufs=1))
        ident = const_pool.tile([P, P], BF16)
        cmasks.make_identity(nc, ident[:])

        persist = ctx.enter_context(tc.tile_pool(name="persist", bufs=1))
        qT = persist.tile([P, DCH, NB], BF16)        # Q_T (d-major)
        kT = persist.tile([P, DCH, NB], BF16)        # K_T (d-major)
        vA = persist.tile([P, NBLK, D], BF16)        # V (n-major)
        mask_t = persist.tile([P, NBLK, WWIN], F32)
        nc.sync.dma_start(mask_t[:], mask_d.rearrange("a p w -> p a w"))

        wpool = ctx.enter_context(tc.tile_pool(name="weights", bufs=1))
        wq = wpool.tile([P, CCH, D], BF16)
        wk = wpool.tile([P, CCH, D], BF16)
        wv = wpool.tile([P, CCH, D], BF16)
        wo = wpool.tile([P, DCH, C], BF16)
        nc.sync.dma_start(wq[:], wq_d.rearrange("(a p) d -> p a d", p=P))
        nc.sync.dma_start(wk[:], wk_d.rearrange("(a p) d -> p a d", p=P))
        nc.sync.dma_start(wv[:], wv_d.rearrange("(a p) d -> p a d", p=P))
        nc.sync.dma_start(wo[:], wo_d.rearrange("(a p) c -> p a c", p=P))

        stream = ctx.enter_context(tc.tile_pool(name="stream", bufs=3))
        tok_pool = ctx.enter_context(tc.tile_pool(name="tokens", bufs=2))
        tokT_pool = ctx.enter_context(tc.tile_pool(name="tokT", bufs=2))
        pos_pool = ctx.enter_context(tc.tile_pool(name="pos", bufs=2))
        otok_pool = ctx.enter_context(tc.tile_pool(name="otok", bufs=2))
        att_pool = ctx.enter_context(tc.tile_pool(name="attn", bufs=2))
        ctxT_pool = ctx.enter_context(tc.tile_pool(name="ctxT", bufs=16))

        # PSUM: 8 banks total -> 4 pools x 2 bufs, shared by size class
        psTR = ctx.enter_context(
            tc.tile_pool(name="psTR", bufs=2, space="PSUM"))  # transposes
        ps512 = ctx.enter_context(
            tc.tile_pool(name="ps512", bufs=2, space="PSUM"))  # V/out_tok
        ps128 = ctx.enter_context(
            tc.tile_pool(name="ps128", bufs=2, space="PSUM"))  # QK/ctx
        psS = ctx.enter_context(
            tc.tile_pool(name="psS", bufs=2, space="PSUM"))   # scores

        def phase_a_proj(i):
            """Stream xb block i (bf16), T-sum, transpose, project Q/K/V."""
            tok = tok_pool.tile([P, C], BF16)
            for j in range(C // A_CC):
                a = stream.tile([P, A_CC, T], BF16, tag="xb")
                nc.sync.dma_start(
                    a[:], xb_h[i * P:(i + 1) * P, j * A_CC:(j + 1) * A_CC, :])
                with nc.allow_low_precision("tokens feed bf16 matmuls"):
                    nc.vector.tensor_reduce(
                        tok[:, j * A_CC:(j + 1) * A_CC], a[:],
                        axis=mybir.AxisListType.X, op=mybir.AluOpType.add)
            tokT = tokT_pool.tile([P, CCH, P], BF16)
            for cc in range(CCH):
                pt = psTR.tile([P, P], BF16, tag="tr")
                nc.tensor.transpose(pt[:], tok[:, cc * P:(cc + 1) * P],
                                    ident[:])
                nc.scalar.copy(tokT[:, cc, :], pt[:])

            ic = slice(i * P, (i + 1) * P)
            posT_t = pos_pool.tile([P, DCH, P], BF16, tag="posT")
            nc.sync.dma_start(
                posT_t[:],
                post_d.rearrange("(a p) n -> p a n", p=P)[:, :, ic])
            for dd in range(DCH):
                for dst, w in ((qT, wq), (kT, wk)):
                    ps = ps128.tile([P, P], F32, tag="qk")
                    for cc in range(CCH):
                        nc.tensor.matmul(
                            ps[:], w[:, cc, dd * P:(dd + 1) * P],
                            tokT[:, cc, :],
                            start=(cc == 0), stop=(cc == CCH - 1))
                    nc.vector.tensor_tensor(
                        dst[:, dd, ic], ps[:], posT_t[:, dd, :],
                        op=mybir.AluOpType.add)
            posn_t = pos_pool.tile([P, D], BF16, tag="posn")
            nc.sync.dma_start(
                posn_t[:], pos_d.rearrange("(a p) d -> p a d", p=P)[:, i, :])
            for dh in range(D // 512):
                ds_ = slice(dh * 512, (dh + 1) * 512)
                ps = ps512.tile([P, 512], F32, tag="v")
                for cc in range(CCH):
                    nc.tensor.matmul(
                        ps[:], tokT[:, cc, :], wv[:, cc, ds_],
                        start=(cc == 0), stop=(cc == CCH - 1))
                nc.vector.tensor_tensor(
                    vA[:, i, ds_], ps[:], posn_t[:, ds_],
                    op=mybir.AluOpType.add)

        def attention(i):
            """Banded attention for block i -> out_tok tile, then phase B."""
            w0 = _w0(i)
            segs = _segments(i)

            sc = psS.tile([P, WWIN], F32)
            for dd in range(DCH):
                nc.tensor.matmul(sc[:], qT[:, dd, i * P:(i + 1) * P],
                                 kT[:, dd, w0:w0 + WWIN],
                                 start=(dd == 0), stop=(dd == DCH - 1))
            msk = att_pool.tile([P, WWIN], F32, tag="msk")
            nc.vector.tensor_tensor(msk[:], sc[:], mask_t[:, i, :],
                                    op=mybir.AluOpType.add)
            att = att_pool.tile([P, WWIN], F32, tag="att")
            den = att_pool.tile([P, 1], F32, tag="den")
            nc.scalar.activation(att[:], msk[:],
                                 mybir.ActivationFunctionType.Exp,
                                 scale=float(SCALE), accum_out=den[:])
            rden = att_pool.tile([P, 1], F32, tag="rden")
            nc.vector.reciprocal(rden[:], den[:])
            attb = att_pool.tile([P, WWIN], BF16, tag="attb")
            nc.vector.tensor_scalar_mul(attb[:], att[:], rden[:])

            # transpose attn segments (32-aligned partition placement)
            attT = []
            for (blk, p0, ln, cofs) in segs:
                pt = psTR.tile([P, P], BF16, tag="tr")
                nc.tensor.transpose(pt[p0:p0 + ln, :],
                                    attb[:, cofs:cofs + ln], ident[:],
                                    tile_position=(0, p0))
                st = att_pool.tile([P, P], BF16, tag="attT_sb", bufs=4)
                nc.scalar.copy(st[p0:p0 + ln, :], pt[p0:p0 + ln, :])
                attT.append(st)

            # ctx_T [d, n] then out_tok [n, c]
            ctxTs = []
            for dd in range(DCH):
                cps = ps128.tile([P, P], F32, tag="qk")
                for k, (blk, p0, ln, cofs) in enumerate(segs):
                    nc.tensor.matmul(
                        cps[:], vA[p0:p0 + ln, blk, dd * P:(dd + 1) * P],
                        attT[k][p0:p0 + ln, :],
                        start=(k == 0), stop=(k == len(segs) - 1),
                        tile_position=(p0, 0))
                cts = ctxT_pool.tile([P, P], BF16)
                nc.scalar.copy(cts[:], cps[:])
                ctxTs.append(cts)
            otok = otok_pool.tile([P, C], F32)
            for ch in range(C // 512):
                cs = slice(ch * 512, (ch + 1) * 512)
                ops = ps512.tile([P, 512], F32, tag="v")
                for dd in range(DCH):
                    nc.tensor.matmul(ops[:], ctxTs[dd][:], wo[:, dd, cs],
                                     start=(dd == 0), stop=(dd == DCH - 1))
                nc.scalar.copy(otok[:, cs], ops[:])

            # Phase B: residual broadcast-add over T
            for j in range(C // B_CC):
                bx = stream.tile([P, B_CC, T], F32, tag="xb")
                nc.sync.dma_start(
                    bx[:], xb_f[i * P:(i + 1) * P, j * B_CC:(j + 1) * B_CC, :])
                ot = otok[:, j * B_CC:(j + 1) * B_CC]
                nc.vector.tensor_tensor(
                    bx[:], bx[:],
                    ot.unsqueeze(-1).broadcast_to((P, B_CC, T)),
                    op=mybir.AluOpType.add)
                nc.sync.dma_start(
                    out_d[i * P:(i + 1) * P, j * B_CC:(j + 1) * B_CC, :],
                    bx[:])

        # software-pipelined emission: attention(i) needs blocks i-1..i+1
        phase_a_proj(0)
        phase_a_proj(1)
        for i in range(NBLK):
            attention(i)
            if i + 2 < NBLK:
                phase_a_proj(i + 2)


_NC = None


def _get_nc():
    global _NC
    if _NC is None:
        _NC = build_nc()
    return _NC


def _prep_in_maps(xb, Wq, Wk, Wv, Wo, pos):
    xb = np.ascontiguousarray(xb, dtype=np.float32)
    bf = ml_dtypes.bfloat16
    xb_h = xb.astype(bf)
    wq_h = (np.asarray(Wq, np.float32) / T).astype(bf)
    wk_h = (np.asarray(Wk, np.float32) / T).astype(bf)
    wv_h = (np.asarray(Wv, np.float32) / T).astype(bf)
    wo_h = np.asarray(Wo, np.float32).astype(bf)
    pos_h = np.asarray(pos, np.float32).astype(bf)
    post_h = np.ascontiguousarray(np.asarray(pos, np.float32).T).astype(bf)
    in_maps = []
    for b in range(B):
        in_maps.append({
            "xb": xb[b], "xbh": xb_h[b],
            "wq": wq_h, "wk": wk_h, "wv": wv_h, "wo": wo_h,
            "pos": pos_h, "post": post_h, "mask": _MASKS,
        })
    return in_maps


def kernel(xb, Wq, Wk, Wv, Wo, pos):
    nc = _get_nc()
    in_maps = _prep_in_maps(xb, Wq, Wk, Wv, Wo, pos)
    res = run_bass_kernel_spmd(nc, in_maps, core_ids=list(range(N_CORES)))
    return np.stack([res.results[b]["out"] for b in range(B)], axis=0)


def run_profiled(xb, Wq, Wk, Wv, Wo, pos, **kw):
    """Like kernel(), but NTFF-profiled; returns (out, BassKernelResults)."""
    import sys, types
    if "antenv.axon_hooks" not in sys.modules:
        try:
            from trn_agent_boot.trn_boot import _ntff_profile_via_ctypes
            hook = _ntff_profile_via_ctypes('/opt/axon/libaxon_pjrt.so')
            mod = types.ModuleType("antenv.axon_hooks")
            mod.get_axon_ntff_profile_hook = lambda: hook
            mod.set_axon_ntff_profile_hook = lambda h: None
            sys.modules["antenv.axon_hooks"] = mod
            import concourse.bass_utils as bu
            bu.upload_artifacts = lambda tmpdir: f"local:{tmpdir}"
        except Exception as e:
            print(f"profiling shim unavailable: {e}")
    nc = _get_nc()
    in_maps = _prep_in_maps(xb, Wq, Wk, Wv, Wo, pos)
    res = run_bass_kernel_spmd(nc, in_maps, core_ids=list(range(N_CORES)),
                               trace=True, **kw)
    out = np.stack([res.results[b]["out"] for b in range(B)], axis=0)
    return out, res


# revision 15
# speedup vs baseline: 1.2607x; 1.2607x over previous
"""Trainium2 Bass kernel for LocalWindowAttention.

Computation (per batch b):
    tokens = xb[b].mean(-1)                    # [NB, C]
    Q/K/V  = tokens @ W{q,k,v} + pos           # [NB, D]
    scores = window-attn over NB (win=9, clamped) with scale 1/sqrt(D)
    ctx    = softmax(scores) @ V_window        # [NB, D]
    out    = xb[b] + (ctx @ Wo)[..., None]     # broadcast over T

Strategy: data-parallel over B across 8 NeuronCores (1 batch/core).
Per core, a single NEFF does:
  Phase A: stream xb (bf16) from HBM, T-sum on DVE -> tokens (bf16),
           PE-transpose -> tokensT.
  Proj:    Q_T/K_T (d-major) and V (n-major) via PE matmuls (bf16),
           + pos adds on DVE.
  Attn:    per 128-row block: banded scores via PE against a 136-wide
           K_T window, clamped-window softmax done exactly via an
           additive log-multiplicity mask + ACT exp (accum_out gives
           the denominator), attn transposed on PE, ctx_T and out_tok
           via PE matmuls.
  Phase B: re-stream xb (fp32), DVE broadcast-add of out_tok over T,
           store result.

The clamped gather of the reference (indices clipped at the edges,
duplicating edge rows) is reproduced exactly by adding log(multiplicity)
to the score of each unique column inside the softmax.
"""

import numpy as np
import ml_dtypes

import concourse.bass as bass
import concourse.mybir as mybir
import concourse.tile as tile
import concourse.bacc as bacc
from concourse import masks as cmasks
from concourse.bass_utils import run_bass_kernel_spmd

# Problem shapes (hardcoded per contest rules)
B, NB, C, T = 8, 1024, 1024, 32
D = 1024
WIN, HALF = 9, 4
P = 128                       # partitions
NBLK = NB // P                # 8 row blocks
CCH = C // P                  # 8 c-chunks
DCH = D // P                  # 8 d-chunks
WWIN = 192                    # window columns per block (32-aligned segs)
SCALE = 1.0 / np.sqrt(D)      # 1/32
A_CC = 256                    # phase-A c-chunk per stream tile
B_CC = 128                    # phase-B c-chunk per stream tile

F32 = mybir.dt.float32
BF16 = mybir.dt.bfloat16

N_CORES = 8


def _w0(i):
    """Window start for block i; chosen so all V-block segments are
    32-aligned (legal matmul tile_positions)."""
    return min(max(i * P - 32, 0), NB - WWIN)


def _build_masks():
    """Per-block additive masks [NBLK, P, WWIN] (pre-divided by SCALE):
    log(multiplicity) on in-band columns (reproduces the reference's
    clamped gather exactly), -1e30 elsewhere."""
    m = np.full((NBLK, P, WWIN), -1e30, np.float32)
    for i in range(NBLK):
        w0 = _w0(i)
        for r in range(P):
            n = i * P + r
            idx = np.clip(n - HALF + np.arange(WIN), 0, NB - 1)
            u, cnt = np.unique(idx, return_counts=True)
            m[i, r, u - w0] = np.log(cnt.astype(np.float64)) / SCALE
    return m


_MASKS = _build_masks()


def _segments(i):
    """V-block segments covering window [w0, w0+WWIN) for block i as
    (blk, p0, ln, cofs): rows [p0, p0+ln) of V block `blk` correspond to
    window columns [cofs, cofs+ln). All splits are 32-aligned so both
    the attn transposes and the banded matmuls get legal tile
    positions."""
    w0 = _w0(i)
    segs = []
    lo, hi = w0, w0 + WWIN
    for blk in range(NBLK):
        b0, b1 = blk * P, (blk + 1) * P
        s0, s1 = max(lo, b0), min(hi, b1)
        if s0 < s1:
            segs.append((blk, s0 - b0, s1 - s0, s0 - w0))
    return segs


def build_nc():
    nc = bacc.Bacc("TRN2", target_bir_lowering=False, debug=False,
                   num_devices=N_CORES)

    xb_f = nc.declare_dram_parameter("xb", [NB, C, T], F32, isOutput=False)
    xb_h = nc.declare_dram_parameter("xbh", [NB, C, T], BF16, isOutput=False)
    wq_d = nc.declare_dram_parameter("wq", [C, D], BF16, isOutput=False)
    wk_d = nc.declare_dram_parameter("wk", [C, D], BF16, isOutput=False)
    wv_d = nc.declare_dram_parameter("wv", [C, D], BF16, isOutput=False)
    wo_d = nc.declare_dram_parameter("wo", [D, C], BF16, isOutput=False)
    pos_d = nc.declare_dram_parameter("pos", [NB, D], BF16, isOutput=False)
    post_d = nc.declare_dram_parameter("post", [D, NB], BF16, isOutput=False)
    mask_d = nc.declare_dram_parameter("mask", [NBLK, P, WWIN], BF16,
                                       isOutput=False)
    out_d = nc.declare_dram_parameter("out", [NB, C, T], F32, isOutput=True)

    with tile.TileContext(nc) as tc:
        _emit(nc, tc, xb_f, xb_h, wq_d, wk_d, wv_d, wo_d, pos_d, post_d,
              mask_d, out_d)

    nc.compile()
    return nc


def _emit(nc, tc, xb_f, xb_h, wq_d, wk_d, wv_d, wo_d, pos_d, post_d,
          mask_d, out_d):
    from contextlib import ExitStack

    with ExitStack() as ctx:
        const_pool = ctx.enter_context(tc.tile_pool(name="const", bufs=1))
        ident = const_pool.tile([P, P], BF16)
        cmasks.make_identity(nc, ident[:])

        persist = ctx.enter_context(tc.tile_pool(name="persist", bufs=1))
        qT = persist.tile([P, DCH, NB], BF16)        # Q_T (d-major)
        kT = persist.tile([P, DCH, NB], BF16)        # K_T (d-major)
        vA = persist.tile([P, NBLK, D], BF16)        # V (n-major)
        mask_t = persist.tile([P, NBLK, WWIN], BF16)
        nc.sync.dma_start(mask_t[:], mask_d.rearrange("a p w -> p a w"))

        wpool = ctx.enter_context(tc.tile_pool(name="weights", bufs=1))
        wq = wpool.tile([P, CCH, D], BF16)
        wk = wpool.tile([P, CCH, D], BF16)
        wv = wpool.tile([P, CCH, D], BF16)
        wo = wpool.tile([P, DCH, C], BF16)
        nc.sync.dma_start(wq[:], wq_d.rearrange("(a p) d -> p a d", p=P))
        nc.sync.dma_start(wk[:], wk_d.rearrange("(a p) d -> p a d", p=P))
        nc.sync.dma_start(wv[:], wv_d.rearrange("(a p) d -> p a d", p=P))
        nc.sync.dma_start(wo[:], wo_d.rearrange("(a p) c -> p a c", p=P))

        stream = ctx.enter_context(tc.tile_pool(name="stream", bufs=4))
        tok_pool = ctx.enter_context(tc.tile_pool(name="tokens", bufs=1))
        tokT_pool = ctx.enter_context(tc.tile_pool(name="tokT", bufs=2))
        pos_pool = ctx.enter_context(tc.tile_pool(name="pos", bufs=1))
        otok_pool = ctx.enter_context(tc.tile_pool(name="otok", bufs=2))
        att_pool = ctx.enter_context(tc.tile_pool(name="attn", bufs=1))
        ctxT_pool = ctx.enter_context(tc.tile_pool(name="ctxT", bufs=12))

        # PSUM: 8 banks total -> 4 pools x 2 bufs, shared by size class
        psTR = ctx.enter_context(
            tc.tile_pool(name="psTR", bufs=2, space="PSUM"))  # transposes
        ps512 = ctx.enter_context(
            tc.tile_pool(name="ps512", bufs=2, space="PSUM"))  # V/out_tok
        ps128 = ctx.enter_context(
            tc.tile_pool(name="ps128", bufs=2, space="PSUM"))  # QK/ctx
        psS = ctx.enter_context(
            tc.tile_pool(name="psS", bufs=2, space="PSUM"))   # scores

        def phase_a_proj(i):
            """Stream xb block i (bf16), T-sum, transpose, project Q/K/V."""
            tok = tok_pool.tile([P, C], BF16)
            for j in range(C // A_CC):
                a = stream.tile([P, A_CC, T], BF16, tag="xb")
                nc.sync.dma_start(
                    a[:], xb_h[i * P:(i + 1) * P, j * A_CC:(j + 1) * A_CC, :])
                # in-place bf16 tree-sum over T: TT adds run in DVE 2x mode
                # (vs 1x for tensor_reduce), halving the mean cost
                for h in (16, 8, 4, 2):
                    nc.vector.tensor_tensor(
                        a[:, :, 0:h], a[:, :, 0:h], a[:, :, h:2 * h],
                        op=mybir.AluOpType.add)
                with nc.allow_low_precision("tokens feed bf16 matmuls"):
                    nc.vector.tensor_reduce(
                        tok[:, j * A_CC:(j + 1) * A_CC], a[:, :, 0:2],
                        axis=mybir.AxisListType.X, op=mybir.AluOpType.add)
            tokT = tokT_pool.tile([P, CCH, P], BF16)
            for cc in range(CCH):
                pt = psTR.tile([P, P], BF16, tag="tr")
                nc.tensor.transpose(pt[:], tok[:, cc * P:(cc + 1) * P],
                                    ident[:])
                nc.scalar.copy(tokT[:, cc, :], pt[:])

            ic = slice(i * P, (i + 1) * P)
            posT_t = pos_pool.tile([P, DCH, P], BF16, tag="posT")
            nc.sync.dma_start(
                posT_t[:],
                post_d.rearrange("(a p) n -> p a n", p=P)[:, :, ic])
            for dd in range(DCH):
                for dst, w in ((qT, wq), (kT, wk)):
                    ps = ps128.tile([P, P], F32, tag="qk")
                    for cc in range(CCH):
                        nc.tensor.matmul(
                            ps[:], w[:, cc, dd * P:(dd + 1) * P],
                            tokT[:, cc, :],
                            start=(cc == 0), stop=(cc == CCH - 1))
                    nc.vector.tensor_tensor(
                        dst[:, dd, ic], ps[:], posT_t[:, dd, :],
                        op=mybir.AluOpType.add)
            posn_t = pos_pool.tile([P, D], BF16, tag="posn")
            nc.sync.dma_start(
                posn_t[:], pos_d.rearrange("(a p) d -> p a d", p=P)[:, i, :])
            for dh in range(D // 512):
                ds_ = slice(dh * 512, (dh + 1) * 512)
                ps = ps512.tile([P, 512], F32, tag="v")
                for cc in range(CCH):
                    nc.tensor.matmul(
                        ps[:], tokT[:, cc, :], wv[:, cc, ds_],
                        start=(cc == 0), stop=(cc == CCH - 1))
                nc.vector.tensor_tensor(
                    vA[:, i, ds_], ps[:], posn_t[:, ds_],
                    op=mybir.AluOpType.add)

        def attention(i):
            """Banded attention for block i -> out_tok tile, then phase B."""
            w0 = _w0(i)
            segs = _segments(i)

            sc = psS.tile([P, WWIN], F32)
            for dd in range(DCH):
                nc.tensor.matmul(sc[:], qT[:, dd, i * P:(i + 1) * P],
                                 kT[:, dd, w0:w0 + WWIN],
                                 start=(dd == 0), stop=(dd == DCH - 1))
            msk = att_pool.tile([P, WWIN], F32, tag="msk")
            nc.vector.tensor_tensor(msk[:], sc[:], mask_t[:, i, :],
                                    op=mybir.AluOpType.add)
            att = att_pool.tile([P, WWIN], F32, tag="att")
            den = att_pool.tile([P, 1], F32, tag="den")
            nc.scalar.activation(att[:], msk[:],
                                 mybir.ActivationFunctionType.Exp,
                                 scale=float(SCALE), accum_out=den[:])
            rden = att_pool.tile([P, 1], F32, tag="rden")
            nc.vector.reciprocal(rden[:], den[:])
            attb = att_pool.tile([P, WWIN], BF16, tag="attb", bufs=2)
            nc.vector.tensor_scalar_mul(attb[:], att[:], rden[:])

            # transpose attn segments (32-aligned partition placement)
            attT = []
            for (blk, p0, ln, cofs) in segs:
                pt = psTR.tile([P, P], BF16, tag="tr")
                nc.tensor.transpose(pt[p0:p0 + ln, :],
                                    attb[:, cofs:cofs + ln], ident[:],
                                    tile_position=(0, p0))
                st = att_pool.tile([P, P], BF16, tag="attT_sb", bufs=4)
                nc.scalar.copy(st[p0:p0 + ln, :], pt[p0:p0 + ln, :])
                attT.append(st)

            # ctx_T [d, n] then out_tok [n, c]
            ctxTs = []
            for dd in range(DCH):
                cps = ps128.tile([P, P], F32, tag="qk")
                for k, (blk, p0, ln, cofs) in enumerate(segs):
                    nc.tensor.matmul(
                        cps[:], vA[p0:p0 + ln, blk, dd * P:(dd + 1) * P],
                        attT[k][p0:p0 + ln, :],
                        start=(k == 0), stop=(k == len(segs) - 1),
                        tile_position=(p0, 0))
                cts = ctxT_pool.tile([P, P], BF16)
                nc.scalar.copy(cts[:], cps[:])
                ctxTs.append(cts)
            otok = otok_pool.tile([P, C], F32)
            for ch in range(C // 512):
                cs = slice(ch * 512, (ch + 1) * 512)
                ops = ps512.tile([P, 512], F32, tag="v")
                for dd in range(DCH):
                    nc.tensor.matmul(ops[:], ctxTs[dd][:], wo[:, dd, cs],
                                     start=(dd == 0), stop=(dd == DCH - 1))
                nc.scalar.copy(otok[:, cs], ops[:])
            return otok

        def phase_b(i, otok):
            """Residual broadcast-add over T for block i."""
            for j in range(C // B_CC):
                bx = stream.tile([P, B_CC, T], F32, tag="xb")
                nc.sync.dma_start(
                    bx[:], xb_f[i * P:(i + 1) * P, j * B_CC:(j + 1) * B_CC, :])
                ot = otok[:, j * B_CC:(j + 1) * B_CC]
                nc.vector.tensor_tensor(
                    bx[:], bx[:],
                    ot.unsqueeze(-1).broadcast_to((P, B_CC, T)),
                    op=mybir.AluOpType.add)
                nc.sync.dma_start(
                    out_d[i * P:(i + 1) * P, j * B_CC:(j + 1) * B_CC, :],
                    bx[:])

        # software-pipelined emission: attention(i) needs blocks i-1..i+1;
        # keep phase-A prefetch ahead of phase-B consumption in trace order
        phase_a_proj(0)
        phase_a_proj(1)
        pend = []
        for i in range(NBLK):
            pend.append((i, attention(i)))
            if i + 2 < NBLK:
                phase_a_proj(i + 2)
            for (bi, ot) in pend:
                phase_b(bi, ot)
            pend = []


_NC = None


def _get_nc():
    global _NC
    if _NC is None:
        _NC = build_nc()
    return _NC


def _prep_in_maps(xb, Wq, Wk, Wv, Wo, pos):
    xb = np.ascontiguousarray(xb, dtype=np.float32)
    bf = ml_dtypes.bfloat16
    xb_h = xb.astype(bf)
    wq_h = (np.asarray(Wq, np.float32) / T).astype(bf)
    wk_h = (np.asarray(Wk, np.float32) / T).astype(bf)
    wv_h = (np.asarray(Wv, np.float32) / T).astype(bf)
    wo_h = np.asarray(Wo, np.float32).astype(bf)
    pos_h = np.asarray(pos, np.float32).astype(bf)
    post_h = np.ascontiguousarray(np.asarray(pos, np.float32).T).astype(bf)
    in_maps = []
    for b in range(B):
        in_maps.append({
            "xb": xb[b], "xbh": xb_h[b],
            "wq": wq_h, "wk": wk_h, "wv": wv_h, "wo": wo_h,
            "pos": pos_h, "post": post_h, "mask": _MASKS.astype(bf),
        })
    return in_maps


def kernel(xb, Wq, Wk, Wv, Wo, pos):
    nc = _get_nc()
    in_maps = _prep_in_maps(xb, Wq, Wk, Wv, Wo, pos)
    res = run_bass_kernel_spmd(nc, in_maps, core_ids=list(range(N_CORES)))
    return np.stack([res.results[b]["out"] for b in range(B)], axis=0)


def run_profiled(xb, Wq, Wk, Wv, Wo, pos, **kw):
    """Like kernel(), but NTFF-profiled; returns (out, BassKernelResults)."""
    import sys, types
    if "antenv.axon_hooks" not in sys.modules:
        try:
            from trn_agent_boot.trn_boot import _ntff_profile_via_ctypes
            hook = _ntff_profile_via_ctypes('/opt/axon/libaxon_pjrt.so')
            mod = types.ModuleType("antenv.axon_hooks")
            mod.get_axon_ntff_profile_hook = lambda: hook
            mod.set_axon_ntff_profile_hook = lambda h: None
            sys.modules["antenv.axon_hooks"] = mod
            import concourse.bass_utils as bu
            bu.upload_artifacts = lambda tmpdir: f"local:{tmpdir}"
        except Exception as e:
            print(f"profiling shim unavailable: {e}")
    nc = _get_nc()
    in_maps = _prep_in_maps(xb, Wq, Wk, Wv, Wo, pos)
    res = run_bass_kernel_spmd(nc, in_maps, core_ids=list(range(N_CORES)),
                               trace=True, **kw)
    out = np.stack([res.results[b]["out"] for b in range(B)], axis=0)
    return out, res


# revision 16
# speedup vs baseline: 1.3922x; 1.1043x over previous
"""Trainium2 Bass kernel for LocalWindowAttention.

Computation (per batch b):
    tokens = xb[b].mean(-1)                    # [NB, C]
    Q/K/V  = tokens @ W{q,k,v} + pos           # [NB, D]
    scores = window-attn over NB (win=9, clamped) with scale 1/sqrt(D)
    ctx    = softmax(scores) @ V_window        # [NB, D]
    out    = xb[b] + (ctx @ Wo)[..., None]     # broadcast over T

Strategy: data-parallel over B across 8 NeuronCores (1 batch/core).
Per core, a single NEFF does:
  Phase A: stream xb (bf16) from HBM, T-sum on DVE -> tokens (bf16),
           PE-transpose -> tokensT.
  Proj:    Q_T/K_T (d-major) and V (n-major) via PE matmuls (bf16),
           + pos adds on DVE.
  Attn:    per 128-row block: banded scores via PE against a 136-wide
           K_T window, clamped-window softmax done exactly via an
           additive log-multiplicity mask + ACT exp (accum_out gives
           the denominator), attn transposed on PE, ctx_T and out_tok
           via PE matmuls.
  Phase B: re-stream xb (fp32), DVE broadcast-add of out_tok over T,
           store result.

The clamped gather of the reference (indices clipped at the edges,
duplicating edge rows) is reproduced exactly by adding log(multiplicity)
to the score of each unique column inside the softmax.
"""

import numpy as np
import ml_dtypes

import concourse.bass as bass
import concourse.mybir as mybir
import concourse.tile as tile
import concourse.bacc as bacc
from concourse import masks as cmasks
from concourse.bass_utils import run_bass_kernel_spmd

# Problem shapes (hardcoded per contest rules)
B, NB, C, T = 8, 1024, 1024, 32
D = 1024
WIN, HALF = 9, 4
P = 128                       # partitions
NBLK = NB // P                # 8 row blocks
CCH = C // P                  # 8 c-chunks
DCH = D // P                  # 8 d-chunks
WWIN = 192                    # window columns per block (32-aligned segs)
SCALE = 1.0 / np.sqrt(D)      # 1/32
A_CC = 128                    # phase-A c-chunk per stream tile
B_CC = 128                    # phase-B c-chunk per stream tile

F32 = mybir.dt.float32
BF16 = mybir.dt.bfloat16
F16 = mybir.dt.float16

N_CORES = 8


def _w0(i):
    """Window start for block i; chosen so all V-block segments are
    32-aligned (legal matmul tile_positions)."""
    return min(max(i * P - 32, 0), NB - WWIN)


def _build_masks():
    """Per-block additive masks [NBLK, P, WWIN] (pre-divided by SCALE):
    log(multiplicity) on in-band columns (reproduces the reference's
    clamped gather exactly), -1e30 elsewhere."""
    m = np.full((NBLK, P, WWIN), -1e30, np.float32)
    for i in range(NBLK):
        w0 = _w0(i)
        for r in range(P):
            n = i * P + r
            idx = np.clip(n - HALF + np.arange(WIN), 0, NB - 1)
            u, cnt = np.unique(idx, return_counts=True)
            m[i, r, u - w0] = np.log(cnt.astype(np.float64)) / SCALE
    return m


_MASKS = _build_masks()


def _segments(i):
    """V-block segments covering window [w0, w0+WWIN) for block i as
    (blk, p0, ln, cofs): rows [p0, p0+ln) of V block `blk` correspond to
    window columns [cofs, cofs+ln). All splits are 32-aligned so both
    the attn transposes and the banded matmuls get legal tile
    positions."""
    w0 = _w0(i)
    segs = []
    lo, hi = w0, w0 + WWIN
    for blk in range(NBLK):
        b0, b1 = blk * P, (blk + 1) * P
        s0, s1 = max(lo, b0), min(hi, b1)
        if s0 < s1:
            segs.append((blk, s0 - b0, s1 - s0, s0 - w0))
    return segs


def build_nc():
    nc = bacc.Bacc("TRN2", target_bir_lowering=False, debug=False,
                   num_devices=N_CORES)

    xb_h = nc.declare_dram_parameter("xbh", [NB, C, T], F16, isOutput=False)
    wq_d = nc.declare_dram_parameter("wq", [C, D], BF16, isOutput=False)
    wk_d = nc.declare_dram_parameter("wk", [C, D], BF16, isOutput=False)
    wv_d = nc.declare_dram_parameter("wv", [C, D], BF16, isOutput=False)
    wo_d = nc.declare_dram_parameter("wo", [D, C], BF16, isOutput=False)
    pos_d = nc.declare_dram_parameter("pos", [NB, D], BF16, isOutput=False)
    post_d = nc.declare_dram_parameter("post", [D, NB], BF16, isOutput=False)
    mask_d = nc.declare_dram_parameter("mask", [NBLK, P, WWIN], BF16,
                                       isOutput=False)
    out_d = nc.declare_dram_parameter("out", [NB, C, T], F32, isOutput=True)

    with tile.TileContext(nc) as tc:
        _emit(nc, tc, xb_h, wq_d, wk_d, wv_d, wo_d, pos_d, post_d,
              mask_d, out_d)

    nc.compile()
    return nc


def _emit(nc, tc, xb_h, wq_d, wk_d, wv_d, wo_d, pos_d, post_d,
          mask_d, out_d):
    from contextlib import ExitStack

    with ExitStack() as ctx:
        const_pool = ctx.enter_context(tc.tile_pool(name="const", bufs=1))
        ident = const_pool.tile([P, P], BF16)
        cmasks.make_identity(nc, ident[:])

        persist = ctx.enter_context(tc.tile_pool(name="persist", bufs=1))
        qT = persist.tile([P, DCH, NB], BF16)        # Q_T (d-major)
        kT = persist.tile([P, DCH, NB], BF16)        # K_T (d-major)
        vA = persist.tile([P, NBLK, D], BF16)        # V (n-major)
        mask_t = persist.tile([P, NBLK, WWIN], BF16)
        nc.sync.dma_start(mask_t[:], mask_d.rearrange("a p w -> p a w"))

        wpool = ctx.enter_context(tc.tile_pool(name="weights", bufs=1))
        wq = wpool.tile([P, CCH, D], BF16)
        wk = wpool.tile([P, CCH, D], BF16)
        wv = wpool.tile([P, CCH, D], BF16)
        wo = wpool.tile([P, DCH, C], BF16)
        nc.sync.dma_start(wq[:], wq_d.rearrange("(a p) d -> p a d", p=P))
        nc.sync.dma_start(wk[:], wk_d.rearrange("(a p) d -> p a d", p=P))
        nc.sync.dma_start(wv[:], wv_d.rearrange("(a p) d -> p a d", p=P))
        nc.sync.dma_start(wo[:], wo_d.rearrange("(a p) c -> p a c", p=P))

        stream = ctx.enter_context(tc.tile_pool(name="stream", bufs=6))
        tok_pool = ctx.enter_context(tc.tile_pool(name="tokens", bufs=1))
        tokT_pool = ctx.enter_context(tc.tile_pool(name="tokT", bufs=2))
        pos_pool = ctx.enter_context(tc.tile_pool(name="pos", bufs=1))
        otok_pool = ctx.enter_context(tc.tile_pool(name="otok", bufs=2))
        att_pool = ctx.enter_context(tc.tile_pool(name="attn", bufs=1))
        ctxT_pool = ctx.enter_context(tc.tile_pool(name="ctxT", bufs=12))

        # PSUM: 8 banks total -> 4 pools x 2 bufs, shared by size class
        psTR = ctx.enter_context(
            tc.tile_pool(name="psTR", bufs=2, space="PSUM"))  # transposes
        ps512 = ctx.enter_context(
            tc.tile_pool(name="ps512", bufs=2, space="PSUM"))  # V/out_tok
        ps128 = ctx.enter_context(
            tc.tile_pool(name="ps128", bufs=2, space="PSUM"))  # QK/ctx
        psS = ctx.enter_context(
            tc.tile_pool(name="psS", bufs=2, space="PSUM"))   # scores

        def phase_a_proj(i):
            """Stream xb block i (bf16), T-sum, transpose, project Q/K/V."""
            tok = tok_pool.tile([P, C], BF16)
            for j in range(C // A_CC):
                a = stream.tile([P, A_CC, T], F16, tag="xb")
                nc.sync.dma_start(
                    a[:], xb_h[i * P:(i + 1) * P, j * A_CC:(j + 1) * A_CC, :])
                # in-place bf16 tree-sum over T: TT adds run in DVE 2x mode
                # (vs 1x for tensor_reduce), halving the mean cost
                for h in (16, 8, 4, 2):
                    nc.vector.tensor_tensor(
                        a[:, :, 0:h], a[:, :, 0:h], a[:, :, h:2 * h],
                        op=mybir.AluOpType.add)
                with nc.allow_low_precision("tokens feed bf16 matmuls"):
                    nc.vector.tensor_reduce(
                        tok[:, j * A_CC:(j + 1) * A_CC], a[:, :, 0:2],
                        axis=mybir.AxisListType.X, op=mybir.AluOpType.add)
            tokT = tokT_pool.tile([P, CCH, P], BF16)
            for cc in range(CCH):
                pt = psTR.tile([P, P], BF16, tag="tr")
                nc.tensor.transpose(pt[:], tok[:, cc * P:(cc + 1) * P],
                                    ident[:])
                nc.scalar.copy(tokT[:, cc, :], pt[:])

            ic = slice(i * P, (i + 1) * P)
            posT_t = pos_pool.tile([P, DCH, P], BF16, tag="posT")
            nc.sync.dma_start(
                posT_t[:],
                post_d.rearrange("(a p) n -> p a n", p=P)[:, :, ic])
            for dd in range(DCH):
                for dst, w in ((qT, wq), (kT, wk)):
                    ps = ps128.tile([P, P], F32, tag="qk")
                    for cc in range(CCH):
                        nc.tensor.matmul(
                            ps[:], w[:, cc, dd * P:(dd + 1) * P],
                            tokT[:, cc, :],
                            start=(cc == 0), stop=(cc == CCH - 1))
                    nc.vector.tensor_tensor(
                        dst[:, dd, ic], ps[:], posT_t[:, dd, :],
                        op=mybir.AluOpType.add)
            posn_t = pos_pool.tile([P, D], BF16, tag="posn")
            nc.sync.dma_start(
                posn_t[:], pos_d.rearrange("(a p) d -> p a d", p=P)[:, i, :])
            for dh in range(D // 512):
                ds_ = slice(dh * 512, (dh + 1) * 512)
                ps = ps512.tile([P, 512], F32, tag="v")
                for cc in range(CCH):
                    nc.tensor.matmul(
                        ps[:], tokT[:, cc, :], wv[:, cc, ds_],
                        start=(cc == 0), stop=(cc == CCH - 1))
                nc.vector.tensor_tensor(
                    vA[:, i, ds_], ps[:], posn_t[:, ds_],
                    op=mybir.AluOpType.add)

        def attention(i):
            """Banded attention for block i -> out_tok tile, then phase B."""
            w0 = _w0(i)
            segs = _segments(i)

            sc = psS.tile([P, WWIN], F32)
            for dd in range(DCH):
                nc.tensor.matmul(sc[:], qT[:, dd, i * P:(i + 1) * P],
                                 kT[:, dd, w0:w0 + WWIN],
                                 start=(dd == 0), stop=(dd == DCH - 1))
            msk = att_pool.tile([P, WWIN], F32, tag="msk")
            nc.vector.tensor_tensor(msk[:], sc[:], mask_t[:, i, :],
                                    op=mybir.AluOpType.add)
            att = att_pool.tile([P, WWIN], F32, tag="att")
            den = att_pool.tile([P, 1], F32, tag="den")
            nc.scalar.activation(att[:], msk[:],
                                 mybir.ActivationFunctionType.Exp,
                                 scale=float(SCALE), accum_out=den[:])
            rden = att_pool.tile([P, 1], F32, tag="rden")
            nc.vector.reciprocal(rden[:], den[:])
            attb = att_pool.tile([P, WWIN], BF16, tag="attb", bufs=2)
            nc.vector.tensor_scalar_mul(attb[:], att[:], rden[:])

            # transpose attn segments (32-aligned partition placement)
            attT = []
            for (blk, p0, ln, cofs) in segs:
                pt = psTR.tile([P, P], BF16, tag="tr")
                nc.tensor.transpose(pt[p0:p0 + ln, :],
                                    attb[:, cofs:cofs + ln], ident[:],
                                    tile_position=(0, p0))
                st = att_pool.tile([P, P], BF16, tag="attT_sb", bufs=4)
                nc.scalar.copy(st[p0:p0 + ln, :], pt[p0:p0 + ln, :])
                attT.append(st)

            # ctx_T [d, n] then out_tok [n, c]
            ctxTs = []
            for dd in range(DCH):
                cps = ps128.tile([P, P], F32, tag="qk")
                for k, (blk, p0, ln, cofs) in enumerate(segs):
                    nc.tensor.matmul(
                        cps[:], vA[p0:p0 + ln, blk, dd * P:(dd + 1) * P],
                        attT[k][p0:p0 + ln, :],
                        start=(k == 0), stop=(k == len(segs) - 1),
                        tile_position=(p0, 0))
                cts = ctxT_pool.tile([P, P], BF16)
                nc.scalar.copy(cts[:], cps[:])
                ctxTs.append(cts)
            otok = otok_pool.tile([P, C], F32)
            for ch in range(C // 512):
                cs = slice(ch * 512, (ch + 1) * 512)
                ops = ps512.tile([P, 512], F32, tag="v")
                for dd in range(DCH):
                    nc.tensor.matmul(ops[:], ctxTs[dd][:], wo[:, dd, cs],
                                     start=(dd == 0), stop=(dd == DCH - 1))
                nc.scalar.copy(otok[:, cs], ops[:])
            return otok

        def phase_b(i, otok):
            """Residual broadcast-add over T for block i."""
            for j in range(C // B_CC):
                bx = stream.tile([P, B_CC, T], F16, tag="xb")
                nc.sync.dma_start(
                    bx[:], xb_h[i * P:(i + 1) * P, j * B_CC:(j + 1) * B_CC, :])
                ot = otok[:, j * B_CC:(j + 1) * B_CC]
                nc.vector.tensor_tensor(
                    bx[:], bx[:],
                    ot.unsqueeze(-1).broadcast_to((P, B_CC, T)),
                    op=mybir.AluOpType.add)
                # SWDGE store casts fp16 -> fp32 inline
                nc.gpsimd.dma_start(
                    out_d[i * P:(i + 1) * P, j * B_CC:(j + 1) * B_CC, :],
                    bx[:])

        # software-pipelined emission: attention(i) needs blocks i-1..i+1;
        # keep phase-A prefetch ahead of phase-B consumption in trace order
        phase_a_proj(0)
        phase_a_proj(1)
        pend = []
        for i in range(NBLK):
            pend.append((i, attention(i)))
            if i + 2 < NBLK:
                phase_a_proj(i + 2)
            for (bi, ot) in pend:
                phase_b(bi, ot)
            pend = []


_NC = None


def _get_nc():
    global _NC
    if _NC is None:
        _NC = build_nc()
    return _NC


def _prep_in_maps(xb, Wq, Wk, Wv, Wo, pos):
    xb = np.ascontiguousarray(xb, dtype=np.float32)
    bf = ml_dtypes.bfloat16
    xb_h = xb.astype(np.float16)
    wq_h = (np.asarray(Wq, np.float32) / T).astype(bf)
    wk_h = (np.asarray(Wk, np.float32) / T).astype(bf)
    wv_h = (np.asarray(Wv, np.float32) / T).astype(bf)
    wo_h = np.asarray(Wo, np.float32).astype(bf)
    pos_h = np.asarray(pos, np.float32).astype(bf)
    post_h = np.ascontiguousarray(np.asarray(pos, np.float32).T).astype(bf)
    in_maps = []
    for b in range(B):
        in_maps.append({
            "xbh": xb_h[b],
            "wq": wq_h, "wk": wk_h, "wv": wv_h, "wo": wo_h,
            "pos": pos_h, "post": post_h, "mask": _MASKS.astype(bf),
        })
    return in_maps


def kernel(xb, Wq, Wk, Wv, Wo, pos):
    nc = _get_nc()
    in_maps = _prep_in_maps(xb, Wq, Wk, Wv, Wo, pos)
    res = run_bass_kernel_spmd(nc, in_maps, core_ids=list(range(N_CORES)))
    return np.stack([res.results[b]["out"] for b in range(B)], axis=0)


def run_profiled(xb, Wq, Wk, Wv, Wo, pos, **kw):
    """Like kernel(), but NTFF-profiled; returns (out, BassKernelResults)."""
    import sys, types
    if "antenv.axon_hooks" not in sys.modules:
        try:
            from trn_agent_boot.trn_boot import _ntff_profile_via_ctypes
            hook = _ntff_profile_via_ctypes('/opt/axon/libaxon_pjrt.so')
            mod = types.ModuleType("antenv.axon_hooks")
            mod.get_axon_ntff_profile_hook = lambda: hook
            mod.set_axon_ntff_profile_hook = lambda h: None
            sys.modules["antenv.axon_hooks"] = mod
            import concourse.bass_utils as bu
            bu.upload_artifacts = lambda tmpdir: f"local:{tmpdir}"
        except Exception as e:
            print(f"profiling shim unavailable: {e}")
    nc = _get_nc()
    in_maps = _prep_in_maps(xb, Wq, Wk, Wv, Wo, pos)
    res = run_bass_kernel_spmd(nc, in_maps, core_ids=list(range(N_CORES)),
                               trace=True, **kw)
    out = np.stack([res.results[b]["out"] for b in range(B)], axis=0)
    return out, res
